# revision 1
# baseline (speedup 1.0000x reference)
"""PeakDetectionLoss on 8 Trainium2 cores.

Sharding: time axis split into 8 segments (one per core), all 10 signal rows
(5 ppg + 5 rppg) on every core. Host pre-pads 5 samples of -inf at the global
edges and hands each core overlapping [128, 2058] windows per row, so the
width-11 sliding max needs no device halo exchange.

v2 layout: per-row work is only the DMA + 4-op sliding-max chain + peak mask
(bf16) + per-partition signal sum + block-max copy (ACT). All quarter-res
math (block peak count/value, slot+position, amplitude filter, prev-peak
scan, gap reciprocals) runs as row-merged [128, 10*512]-wide single ops, and
the cummax-style previous-peak scan (tensor_tensor_scan, min) replaces the
pairwise gap tree. Block positions are generated on-device with iota (core
offset is added on the host during stitching), per-row stats cross partitions
via a ones-matmul and cross cores via one [1,30] AllReduce.
"""
import os
import sys

for _p in ("/opt/trn_rl_repo", "/root/.axon_site/_ro/trn_rl_repo"):
    if _p not in sys.path:
        sys.path.append(_p)

import numpy as np

N = 5
L = 2097152
C = 8
SEG = L // C            # 262144
P = 128
PW = SEG // P           # 2048
NB = PW // 4            # 512
TILE_W = PW + 10        # 2058
R = 2 * N               # 10 rows per core
RB = R * NB             # 5120 blocks per partition
BIG = np.float32(1.0e30)

_STATE = {}


def _build_program():
    from concourse import bacc, tile, mybir
    from concourse.alu_op_type import AluOpType as op

    stage = int(os.environ.get("KSTAGE", "99"))
    krepeat = int(os.environ.get("KREPEAT", "1"))
    f32 = mybir.dt.float32
    bf16 = mybir.dt.bfloat16
    X = mybir.AxisListType.X
    nc = bacc.Bacc("TRN2", target_bir_lowering=False, debug=False, num_devices=C)

    xin = nc.dram_tensor("xin", [R, P, TILE_W], f32, kind="ExternalInput")
    summ = nc.dram_tensor("summ", [P, 40], f32, kind="ExternalOutput")

    with tile.TileContext(nc) as tc:
        with (
            tc.tile_pool(name="sb", bufs=1) as sb,
            tc.tile_pool(name="dram", bufs=1, space="DRAM") as dram,
            tc.tile_pool(name="ps", bufs=1, space="PSUM") as ps,
        ):
            ones = sb.tile([P, 1], f32, tag="ones")
            stats = sb.tile([P, 3 * R], f32, tag="stats")
            m1all = sb.tile([P, R * PW], bf16, tag="m1all")
            bidx0 = sb.tile([P, NB], f32, tag="bidx0")
            arsb = sb.tile([1, 3 * R], f32, tag="arsb")
            arst = sb.tile([1, 3 * R], f32, tag="arst")
            trec = sb.tile([1, R], f32, tag="trec")
            tmean = sb.tile([1, R], f32, tag="tmean")
            tthr = sb.tile([1, R], f32, tag="tthr")
            tbc = sb.tile([P, R], f32, tag="tbc")
            summ_sb = sb.tile([P, 40], f32, tag="summ_sb")
            M2 = sb.tile([P, PW + 9], f32, tag="M2")
            M4 = sb.tile([P, PW + 7], f32, tag="M4")
            Wt = sb.tile([P, PW], f32, tag="Wt")

            ar_in = dram.tile([1, 3 * R], f32)
            ar_out = dram.tile([1, 3 * R], f32)
            psum_t = ps.tile([1, 3 * R], f32)

            xin_ap = xin.ap()
            nc.vector.memset(ones, 1.0)
            # bidx0[p, b] = p*PW + 4*b  (values < 2^18, exact in f32)
            nc.gpsimd.iota(bidx0, pattern=[[4, NB]], base=0,
                           channel_multiplier=PW,
                           allow_small_or_imprecise_dtypes=True)
            if stage < 99:
                nc.vector.memset(summ_sb, 0.0)
                nc.vector.memset(stats, 0.0)

            for rep in range(krepeat):
                # ---- stage A: 2-rows-per-DMA + sliding-max + mask + row sums
                for r in range(R if stage >= 1 else 0):
                    if r % 2 == 0:
                        xt2 = sb.tile([P, 2 * TILE_W], f32, tag="xt", bufs=2,
                                      name=f"xt{rep}_{r}")
                        nc.sync.dma_start(
                            xt2.rearrange("p (g w) -> p g w", g=2),
                            xin_ap[r:r + 2].rearrange("g p w -> p g w"))
                    xt = xt2[:, (r % 2) * TILE_W:(r % 2 + 1) * TILE_W]
                    # sliding max chain (window 11, centered at xt[:, j+5])
                    nc.vector.tensor_tensor(
                        out=M2, in0=xt[:, 0:2057], in1=xt[:, 1:2058], op=op.max)
                    nc.vector.tensor_tensor(
                        out=M4, in0=M2[:, 0:2055], in1=M2[:, 2:2057], op=op.max)
                    M8 = M2[:, 0:2051]  # M2 storage reused for M8
                    nc.vector.tensor_tensor(
                        out=M8, in0=M4[:, 0:2051], in1=M4[:, 4:2055], op=op.max)
                    nc.vector.tensor_tensor(
                        out=Wt, in0=M8[:, 0:PW], in1=M2[:, 3:3 + PW], op=op.max)

                    xc = xt[:, 5:5 + PW]
                    # m1 = (x == window max), bf16 {0,1}
                    nc.vector.scalar_tensor_tensor(
                        out=m1all[:, r * PW:(r + 1) * PW], in0=xc, scalar=0.0,
                        op0=op.bypass, in1=Wt, op1=op.is_ge)
                    # per-partition signal sum -> stats Sx column
                    nc.vector.tensor_reduce(
                        out=stats[:, 3 * r + 1:3 * r + 2], in_=xc,
                        axis=X, op=op.add)
                    # aligned block-4 max (free view of M4) -> persistent B4all
                    if r == 0:
                        B4all = sb.tile([P, RB], f32, tag="big", bufs=5, name=f"B4all{rep}")
                    nc.scalar.copy(
                        B4all[:, r * NB:(r + 1) * NB], M4[:, 5:2052:4])

                if stage >= 2:
                    # ---- stage A2: row-merged quarter-res math
                    m1b = m1all.rearrange("p (n k) -> p n k", k=4)
                    n1all = sb.tile([P, RB], f32, tag="big", bufs=5, name=f"n1all{rep}")
                    nc.vector.tensor_reduce(out=n1all, in_=m1b, axis=X, op=op.add)
                    # npk per row -> stats cols 0,3,6,...
                    n13 = n1all.rearrange("p (r b) -> p r b", r=R)
                    nc.vector.tensor_reduce(
                        out=stats[:, 0:3 * R:3], in_=n13, axis=X, op=op.add)
                    bnall = sb.tile([P, RB], f32, tag="big", bufs=5, name=f"bnall{rep}")
                    nc.vector.tensor_tensor(
                        out=bnall, in0=B4all, in1=n1all, op=op.mult)
                    bn3 = bnall.rearrange("p (r b) -> p r b", r=R)
                    nc.vector.tensor_reduce(
                        out=stats[:, 2:3 * R:3], in_=bn3, axis=X, op=op.add)

                    # cross-partition reduce + cross-core AllReduce (kick early)
                    nc.tensor.matmul(
                        out=psum_t[0:1, :], lhsT=ones, rhs=stats,
                        start=True, stop=True)
                    nc.scalar.copy(arst, psum_t[0:1, :])
                    nc.sync.dma_start(ar_in, arst)
                    nc.gpsimd.collective_compute(
                        "AllReduce", op.add, replica_groups=[list(range(C))],
                        ins=[ar_in.opt()], outs=[ar_out.opt()])
                    nc.sync.dma_start(arsb, ar_out)

                    # overlap with collective: block value + position
                    aBall = sb.tile([P, RB], f32, tag="big", bufs=5, name=f"aBall{rep}")
                    nc.vector.tensor_scalar(
                        out=aBall, in0=n1all, scalar1=float(BIG),
                        scalar2=float(-BIG), op0=op.mult, op1=op.add)
                    B4m = sb.tile([P, RB], f32, tag="big", bufs=5, name=f"B4m{rep}")
                    nc.vector.tensor_tensor(
                        out=B4m, in0=bnall, in1=aBall, op=op.add)
                    m14 = m1all.rearrange("p (r b k) -> p r b k", r=R, k=4)
                    bidxb = bidx0.unsqueeze(1).broadcast_to([P, R, NB])
                    t1 = sb.tile([P, RB], f32, tag="big", bufs=5, name=f"t1_{rep}")
                    t13 = t1.rearrange("p (r b) -> p r b", r=R)
                    nc.vector.scalar_tensor_tensor(
                        out=t13, in0=m14[:, :, :, 2], scalar=2.0, op0=op.mult,
                        in1=bidxb, op1=op.add)
                    t2 = sb.tile([P, RB], f32, tag="big", bufs=5, name=f"t2_{rep}")
                    t23 = t2.rearrange("p (r b) -> p r b", r=R)
                    nc.vector.tensor_tensor(
                        out=t23, in0=m14[:, :, :, 1], in1=t13, op=op.add)
                    posall = sb.tile([P, RB], f32, tag="big", bufs=5, name=f"posall{rep}")
                    pos3 = posall.rearrange("p (r b) -> p r b", r=R)
                    nc.vector.scalar_tensor_tensor(
                        out=pos3, in0=m14[:, :, :, 3], scalar=3.0, op0=op.mult,
                        in1=t23, op1=op.add)

                if stage >= 3:
                    # threshold t_r = Sx_r/(2L) + 0.5*sv_r/npk_r (raw-space)
                    a_npk = arsb[0:1, 0:3 * R:3]
                    a_sx = arsb[0:1, 1:3 * R:3]
                    a_sv = arsb[0:1, 2:3 * R:3]
                    nc.vector.reciprocal(out=trec[0:1, :], in_=a_npk)
                    nc.vector.scalar_tensor_tensor(
                        out=tmean[0:1, :], in0=trec[0:1, :], scalar=0.5,
                        op0=op.mult, in1=a_sv, op1=op.mult)
                    nc.vector.scalar_tensor_tensor(
                        out=tthr[0:1, :], in0=a_sx, scalar=0.5 / L, op0=op.mult,
                        in1=tmean[0:1, :], op1=op.add)
                    nc.gpsimd.partition_broadcast(tbc, tthr[0:1, :])

                if stage >= 4:
                    # ---- stage C: amplitude filter + prev-peak scan + gaps
                    B4m3 = B4m.rearrange("p (r b) -> p r b", r=R)
                    tbcb = tbc.unsqueeze(-1).broadcast_to([P, R, NB])
                    notv = sb.tile([P, RB], f32, tag="big", bufs=5, name=f"notv{rep}")
                    nv3 = notv.rearrange("p (r b) -> p r b", r=R)
                    nc.vector.tensor_tensor(
                        out=nv3, in0=B4m3, in1=tbcb, op=op.is_le)
                    nc.vector.tensor_reduce(
                        out=summ_sb[:, 30:40], in_=nv3, axis=X, op=op.add)
                    Bt = sb.tile([P, RB], f32, tag="big", bufs=5, name=f"Bt{rep}")
                    nc.vector.scalar_tensor_tensor(
                        out=Bt, in0=notv, scalar=float(BIG), op0=op.mult,
                        in1=posall, op1=op.add)
                    Ct = sb.tile([P, RB], f32, tag="big", bufs=5, name=f"Ct{rep}")
                    nc.vector.scalar_tensor_tensor(
                        out=Ct, in0=notv, scalar=float(BIG), op0=op.mult,
                        in1=posall, op1=op.subtract)
                    B3 = Bt.rearrange("p (r b) -> p r b", r=R)
                    C3 = Ct.rearrange("p (r b) -> p r b", r=R)
                    nc.vector.tensor_reduce(
                        out=summ_sb[:, 0:R], in_=B3, axis=X, op=op.min)
                    nc.vector.tensor_reduce(
                        out=summ_sb[:, R:2 * R], in_=C3, axis=X, op=op.min)
                    # previous-peak scan: prefix-min of Ct per row
                    pneg = sb.tile([P, RB], f32, tag="big", bufs=5, name=f"pneg{rep}")
                    for r in range(R):
                        nc.vector.tensor_tensor_scan(
                            out=pneg[:, r * NB:(r + 1) * NB],
                            data0=Ct[:, r * NB:(r + 1) * NB],
                            data1=Ct[:, r * NB:(r + 1) * NB],
                            initial=float(BIG), op0=op.min, op1=op.min)
                    pn3 = pneg.rearrange("p (r b) -> p r b", r=R)
                    gap = sb.tile([P, RB], f32, tag="big", bufs=5, name=f"gap{rep}")
                    g3 = gap.rearrange("p (r b) -> p r b", r=R)
                    nc.vector.tensor_tensor(
                        out=g3[:, :, 1:NB], in0=B3[:, :, 1:NB],
                        in1=pn3[:, :, 0:NB - 1], op=op.add)
                    inv = sb.tile([P, RB], f32, tag="big", bufs=5, name=f"inv{rep}")
                    i3 = inv.rearrange("p (r b) -> p r b", r=R)
                    nc.vector.reciprocal(
                        out=i3[:, :, 1:NB], in_=g3[:, :, 1:NB])
                    nc.vector.tensor_reduce(
                        out=summ_sb[:, 20:30], in_=i3[:, :, 1:NB],
                        axis=X, op=op.add)
                nc.sync.dma_start(summ.ap(), summ_sb)

    nc.compile()
    return nc


def _get_runner():
    """Build once; return fn(in_maps) -> list of per-core {name: np.ndarray}."""
    if "runner" in _STATE:
        return _STATE["runner"]

    import jax
    from jax.sharding import Mesh, PartitionSpec
    from jax.experimental.shard_map import shard_map
    from concourse import bass2jax, mybir

    nc = _build_program()
    bass2jax.install_neuronx_cc_hook()

    partition_name = (
        nc.partition_id_tensor.name if nc.partition_id_tensor else None
    )
    in_names, out_names, out_avals, zero_outs = [], [], [], []
    for alloc in nc.m.functions[0].allocations:
        if not isinstance(alloc, mybir.MemoryLocationSet):
            continue
        name = alloc.memorylocations[0].name
        if alloc.kind == "ExternalInput":
            if name != partition_name:
                in_names.append(name)
        elif alloc.kind == "ExternalOutput":
            out_names.append(name)
            shape = tuple(alloc.tensor_shape)
            dtype = mybir.dt.np(alloc.dtype)
            out_avals.append(jax.core.ShapedArray(shape, dtype))
            zero_outs.append(np.zeros(shape, dtype))
    n_params = len(in_names)
    n_outs = len(out_avals)
    all_names = in_names + out_names
    if partition_name is not None:
        all_names = all_names + [partition_name]

    def _body(*args):
        operands = list(args)
        if partition_name is not None:
            operands.append(bass2jax.partition_id_tensor())
        outs = bass2jax._bass_exec_p.bind(
            *operands,
            out_avals=tuple(out_avals),
            in_names=tuple(all_names),
            out_names=tuple(out_names),
            lowering_input_output_aliases=(),
            sim_require_finite=False,
            sim_require_nnan=False,
            nc=nc,
        )
        return tuple(outs)

    devices = jax.devices()[:C]
    assert len(devices) == C, f"need {C} devices, have {len(jax.devices())}"
    mesh = Mesh(np.asarray(devices), ("core",))
    donate = tuple(range(n_params, n_params + n_outs))
    sharded = jax.jit(
        shard_map(
            _body, mesh=mesh,
            in_specs=(PartitionSpec("core"),) * (n_params + n_outs),
            out_specs=(PartitionSpec("core"),) * n_outs,
            check_rep=False,
        ),
        donate_argnums=donate,
        keep_unused=True,
    )

    def run(in_maps):
        concat_in = [
            np.concatenate([np.asarray(m[nm]) for m in in_maps], axis=0)
            for nm in in_names
        ]
        concat_zeros = [
            np.zeros((C * z.shape[0], *z.shape[1:]), z.dtype) for z in zero_outs
        ]
        out_arrs = sharded(*concat_in, *concat_zeros)
        return [
            {nm: np.asarray(out_arrs[i]).reshape(C, *out_avals[i].shape)[c]
             for i, nm in enumerate(out_names)}
            for c in range(C)
        ]

    run.in_names = in_names
    run.out_names = out_names
    run.sharded = sharded
    run.zero_outs = zero_outs
    _STATE["runner"] = run
    return run


def make_in_maps(rppg, ppg):
    sigs = np.concatenate(
        [np.asarray(ppg, np.float32).reshape(N, L),
         np.asarray(rppg, np.float32).reshape(N, L)], axis=0)
    padded = np.full((R, L + 10), -np.inf, np.float32)
    padded[:, 5:5 + L] = sigs
    win = np.lib.stride_tricks.sliding_window_view(padded, TILE_W, axis=1)
    in_maps = []
    for c in range(C):
        xin_c = np.ascontiguousarray(win[:, c * SEG:c * SEG + SEG:PW, :])
        in_maps.append({"xin": xin_c})
    return in_maps


def stitch(results, fs):
    summ = np.stack([results[c]["summ"] for c in range(C)])  # [C, 128, 40]
    offs = np.repeat(np.arange(C) * SEG, P).astype(np.float64)
    hr = np.zeros(R)
    for r in range(R):
        f = summ[:, :, r].reshape(-1).astype(np.float64) + offs
        g = -summ[:, :, R + r].reshape(-1).astype(np.float64) + offs
        s = summ[:, :, 2 * R + r].astype(np.float64).sum()
        n = (512.0 - summ[:, :, 3 * R + r].astype(np.float64)).sum()
        ne = f < float(BIG) / 2
        fs_, gs_ = f[ne], g[ne]
        s += (1.0 / (fs_[1:] - gs_[:-1])).sum()
        hr[r] = 60.0 * float(fs) * s / (n - 1.0)
    return np.float32(np.mean(np.abs(hr[0:N] - hr[N:R]) / hr[0:N]))


def kernel(rppg, ppg, fs, epoch):
    run = _get_runner()
    results = run(make_in_maps(rppg, ppg))
    return stitch(results, fs)

